# revision 21
# baseline (speedup 1.0000x reference)
"""Trainium2 Bass kernel for nn_Concat_26147760898611.

Mean-pool over the word dim of article_concat [256, 2048, 300] and
options_concat [256, 64, 300], concat features -> [256, 600].

Sharding: pure data parallel over batch across 8 NeuronCores
(32 batches per core). Per core:
  - each article batch [2048, 300] is DMA'd as one 2.46 MB transfer into
    an SBUF tile [128 partitions, 16 words, 300 feat] where partition p
    holds 16 *consecutive* words (fully contiguous 19.2 KB per
    partition -> line-rate DMA).
  - the word axis is folded FOLD_K times on the VectorEngine (fp32-exact
    adds); the surviving chunks are reduced across the partition dim on
    the TensorEngine with a ones-selector stationary operand whose
    single ones-column routes the sum into PSUM row b of a [32, 300]
    accumulator.
  - options: same tile shape; partition p holds 16 consecutive words of
    batch p//4, one block-selector matmul per surviving chunk reduces
    all 32 batches at once.
  - the selector columns carry the 1/n_words scale (exact powers of two),
    so PSUM holds the final means; the DVE copies them into the [32, 600]
    output tile (no ScalarEngine pass).
  - a burst of dummy matmuls at kernel start warms the PE HAM clock
    gate (1.2 -> 2.4 GHz) before real data lands.
  - the options + first article DMA triggers are issued before the sel
    loads so the HBM stream starts at the earliest DIRECT2D slot.
  - the last three batches are split into progressively finer pieces
    (solver: tailsolve.py) so the post-last-DMA tail (fold + matmul +
    copy + store) is short; the final two pieces skip the DVE fold and
    go straight to the PE.
  - ALL DMAs are triggered from the Sync (SP) queue: triggering any DMA
    from another engine puts it on a second hardware ring, which was
    measured to degrade descriptor processing on all 16 DMA queues.

Self-contained: hardcodes all shapes; no file reads.
"""

import numpy as np

N_CORES = 8
B = 256  # full batch
BC = B // N_CORES  # 32 batches per core
DIM = 300
AW = 2048  # article words per batch
OW = 64  # options words per batch
P = 128  # SBUF partitions
AWP = AW // P  # 16 article words per partition
FOLD_K = 2  # DVE fold levels before the PE reduction
TAPER_FOLD_K = 3  # deeper fold for the last article batches (fewer cold
TAPER_START = 99  # PE passes in the tail; disabled (DVE was the tail gate)
# final three batches in progressively finer pieces with fold depths from
# the analytic stream->DVE->PE schedule solver (tailsolve.py)
TAIL_PIECES = [
    # (batch, word_offset, nch, fold_k)
    (BC - 3, 0, 8, 2),
    (BC - 3, 8, 8, 2),
    (BC - 2, 0, 4, 2),
    (BC - 2, 4, 4, 2),
    (BC - 2, 8, 4, 2),
    (BC - 2, 12, 4, 2),
    (BC - 1, 0, 2, 1),
    (BC - 1, 2, 2, 1),
    (BC - 1, 4, 2, 1),
    (BC - 1, 6, 2, 1),
    (BC - 1, 8, 2, 1),
    (BC - 1, 10, 2, 1),
    (BC - 1, 12, 2, 0),
    (BC - 1, 14, 1, 0),
    (BC - 1, 15, 1, 0),
]
DATA_BUFS = 6
FOLD_BUFS = 3
WARMUP_MMS = 12
# float32r would stream the PE in one pass (vs fp32's two half-speed
# passes) but walrus requires the full producer chain to round to f32r
# and rejects this program; kept off.
USE_F32R = False

_CACHE = {}


def _build_nc():
    import concourse.bacc as bacc
    import concourse.mybir as mybir
    import concourse.tile as tile

    f32 = mybir.dt.float32
    f32mm = mybir.dt.float32r if USE_F32R else f32
    nc = bacc.Bacc("TRN2", target_bir_lowering=False, debug=False)

    art = nc.dram_tensor("article", [BC, AW, DIM], f32, kind="ExternalInput")
    opt = nc.dram_tensor("options", [BC, OW, DIM], f32, kind="ExternalInput")
    sel_a = nc.dram_tensor("sel_a", [P, 2 * BC - 1], f32mm, kind="ExternalInput")
    sel_o = nc.dram_tensor("sel_o", [P, BC], f32mm, kind="ExternalInput")
    out = nc.dram_tensor("out", [BC, 2 * DIM], f32, kind="ExternalOutput")

    # [BC, 128, 16, 300]: partition p <- words p*16 .. p*16+15 (contiguous)
    art_r = art.ap().rearrange("b (p w) f -> b p w f", p=P)
    # per-partition word views: [128, 16, 300] per batch
    art_pw = [art.ap()[b].rearrange("(p w) f -> p w f", p=P) for b in range(BC)]
    # [128, 16, 300]: partition p <- 16 consecutive words of batch p//4
    opt_r = opt.ap().rearrange("b (s q) f -> (b s) q f", s=P // BC)

    with tile.TileContext(nc) as tc:
        with (
            tc.tile_pool(name="const", bufs=1) as cpool,
            tc.tile_pool(name="data", bufs=DATA_BUFS) as dpool,
            tc.tile_pool(name="fold", bufs=FOLD_BUFS) as fpool,
            tc.tile_pool(name="outp", bufs=1) as opool,
            tc.tile_pool(name="psum", bufs=1, space="PSUM") as ppool,
        ):
            # data triggers first so the HBM stream starts at the earliest
            # possible DIRECT2D slot; sel loads are tiny and not needed
            # until the first matmul (~13us in)
            t_opt = dpool.tile([P, AWP, DIM], f32, tag="data")
            nc.sync.dma_start(t_opt[:], opt_r)
            t_b0 = dpool.tile([P, AWP, DIM], f32, tag="data")
            nc.sync.dma_start(t_b0[:], art_r[0])
            # NOTE: every dma_start in this kernel goes through nc.sync.
            # Triggering any DMA from another engine queue (Activation,
            # Vector, ...) puts it on a second hardware ring and measurably
            # degrades Q_I descriptor processing on ALL 16 queues for the
            # whole run (+20% on some queues; ~47us end-to-end).
            sel_a_t = cpool.tile([P, 2 * BC - 1], f32mm, tag="sel_a")
            nc.sync.dma_start(sel_a_t[:], sel_a.ap()[:])
            sel_o_t = cpool.tile([P, BC], f32mm, tag="sel_o")
            nc.sync.dma_start(sel_o_t[:], sel_o.ap()[:])

            psum_a = ppool.tile([BC, DIM], f32, tag="psum_a")
            psum_b = ppool.tile([BC, DIM], f32, tag="psum_b")
            psum_w = ppool.tile([BC, 2 * BC - 1], f32, tag="psum_w")

            # PE warmup: flip the HAM clock gate to 2.4 GHz before the
            # first data tile lands. Results are never read.
            for _ in range(WARMUP_MMS):
                nc.tensor.matmul(
                    psum_w[:], sel_o_t[:], sel_a_t[:], start=True, stop=True
                )

            out_t = opool.tile([BC, 2 * DIM], f32, tag="out")

            def reduce_block(src_ap, nch, sel_ap, psum, first, last,
                             fold_k=FOLD_K, t=None):
                if t is None:
                    t = dpool.tile([P, nch, DIM], f32, tag="data")
                    nc.sync.dma_start(t[:], src_ap)
                cur, n = t, nch
                for lvl in range(fold_k):
                    if n == 1:
                        break
                    n //= 2
                    nxt = fpool.tile([P, n, DIM], f32, tag=f"fold{lvl}_{nch}")
                    nc.vector.tensor_add(nxt[:], cur[:, 0:n, :], cur[:, n : 2 * n, :])
                    cur = nxt
                for j in range(n):
                    nc.tensor.matmul(
                        psum[:],
                        sel_ap,
                        cur[:, j, :],
                        start=(first and j == 0),
                        stop=(last and j == n - 1),
                    )

            # options first; drain its psum into the output tile early
            # (selector columns carry 1/n, so PSUM already holds the mean)
            reduce_block(None, AWP, sel_o_t[:], psum_b, True, True, t=t_opt)
            nc.vector.tensor_copy(out_t[:, DIM : 2 * DIM], psum_b[:])

            for b in range(BC - 3):
                reduce_block(
                    None if b == 0 else art_r[b],
                    AWP,
                    sel_a_t[:, BC - 1 - b : 2 * BC - 1 - b],
                    psum_a,
                    b == 0,
                    False,
                    fold_k=TAPER_FOLD_K if b >= TAPER_START else FOLD_K,
                    t=t_b0 if b == 0 else None,
                )
            # final batches in shrinking tiles -> the very last DMAs are
            # tiny and their fold+matmul tails are short
            n_pieces = len(TAIL_PIECES)
            for i, (b, w0, nch, fk) in enumerate(TAIL_PIECES):
                reduce_block(
                    art_pw[b][:, w0 : w0 + nch, :],
                    nch,
                    sel_a_t[:, BC - 1 - b : 2 * BC - 1 - b],
                    psum_a,
                    False,
                    i == n_pieces - 1,
                    fold_k=fk,
                )

            nc.vector.tensor_copy(out_t[:, 0:DIM], psum_a[:])
            nc.sync.dma_start(out.ap()[:], out_t[:])

    nc.compile()
    return nc


def get_nc():
    if "nc" not in _CACHE:
        _CACHE["nc"] = _build_nc()
    return _CACHE["nc"]


def _sel_arrays():
    # selector columns carry the mean scale (exact powers of two)
    sel_a = np.zeros((P, 2 * BC - 1), np.float32)
    sel_a[:, BC - 1] = 1.0 / AW
    sel_o = np.zeros((P, BC), np.float32)
    sel_o[np.arange(P), np.arange(P) // (P // BC)] = 1.0 / OW
    return sel_a, sel_o


def make_in_maps(article, options):
    article = np.ascontiguousarray(np.asarray(article, dtype=np.float32))
    options = np.ascontiguousarray(np.asarray(options, dtype=np.float32))
    assert article.shape == (B, AW, DIM), article.shape
    assert options.shape == (B, OW, DIM), options.shape
    sel_a, sel_o = _sel_arrays()
    return [
        {
            "article": article[i * BC : (i + 1) * BC],
            "options": options[i * BC : (i + 1) * BC],
            "sel_a": sel_a,
            "sel_o": sel_o,
        }
        for i in range(N_CORES)
    ]


def run_sharded(article, options, **spmd_kwargs):
    from concourse.bass_utils import run_bass_kernel_spmd

    nc = get_nc()
    in_maps = make_in_maps(article, options)
    res = run_bass_kernel_spmd(nc, in_maps, list(range(N_CORES)), **spmd_kwargs)
    full = np.concatenate(
        [res.results[i]["out"] for i in range(N_CORES)], axis=0
    ).astype(np.float32)
    return full, res


def kernel(article_concat, options_concat):
    full, _ = run_sharded(article_concat, options_concat)
    return full



# revision 22
# speedup vs baseline: 1.1897x; 1.1897x over previous
"""Trainium2 Bass kernel for nn_Concat_26147760898611.

Mean-pool over the word dim of article_concat [256, 2048, 300] and
options_concat [256, 64, 300], concat features -> [256, 600].

Sharding: pure data parallel over batch across 8 NeuronCores
(32 batches per core). Per core:
  - each article batch [2048, 300] is DMA'd as one 2.46 MB transfer into
    an SBUF tile [128 partitions, 16 words, 300 feat] where partition p
    holds 16 *consecutive* words (fully contiguous 19.2 KB per
    partition -> line-rate DMA).
  - the word axis is folded FOLD_K times on the VectorEngine (fp32-exact
    adds); the surviving chunks are reduced across the partition dim on
    the TensorEngine with a ones-selector stationary operand whose
    single ones-column routes the sum into PSUM row b of a [32, 300]
    accumulator.
  - options: same tile shape; partition p holds 16 consecutive words of
    batch p//4, one block-selector matmul per surviving chunk reduces
    all 32 batches at once.
  - the selector columns carry the 1/n_words scale (exact powers of two),
    so PSUM holds the final means; the DVE copies them into the [32, 600]
    output tile (no ScalarEngine pass).
  - a burst of dummy matmuls at kernel start warms the PE HAM clock
    gate (1.2 -> 2.4 GHz) before real data lands.
  - the options + first article DMA triggers are issued before the sel
    loads so the HBM stream starts at the earliest DIRECT2D slot.
  - the last three batches are split into progressively finer pieces
    (solver: tailsolve.py) so the post-last-DMA tail (fold + matmul +
    copy + store) is short; the final two pieces skip the DVE fold and
    go straight to the PE.
  - ALL DMAs are triggered from the Sync (SP) queue: triggering any DMA
    from another engine puts it on a second hardware ring, which was
    measured to degrade descriptor processing on all 16 DMA queues.

Self-contained: hardcodes all shapes; no file reads.
"""

import numpy as np

N_CORES = 8
B = 256  # full batch
BC = B // N_CORES  # 32 batches per core
DIM = 300
AW = 2048  # article words per batch
OW = 64  # options words per batch
P = 128  # SBUF partitions
AWP = AW // P  # 16 article words per partition
FOLD_K = 2  # DVE fold levels before the PE reduction
TAPER_FOLD_K = 3  # deeper fold for the last article batches (fewer cold
TAPER_START = 99  # PE passes in the tail; disabled (DVE was the tail gate)
# final three batches in progressively finer pieces with fold depths from
# the analytic stream->DVE->PE schedule solver (tailsolve.py)
TAIL_PIECES = [
    # (batch, word_offset, nch, fold_k)
    (BC - 3, 0, 8, 2),
    (BC - 3, 8, 8, 2),
    (BC - 2, 0, 4, 2),
    (BC - 2, 4, 4, 2),
    (BC - 2, 8, 4, 2),
    (BC - 2, 12, 4, 2),
    (BC - 1, 0, 2, 1),
    (BC - 1, 2, 2, 1),
    (BC - 1, 4, 2, 1),
    (BC - 1, 6, 2, 1),
    (BC - 1, 8, 2, 1),
    (BC - 1, 10, 2, 1),
    (BC - 1, 12, 2, 0),
    (BC - 1, 14, 2, 0),
]
DATA_BUFS = 6
FOLD_BUFS = 3
WARMUP_MMS = 12
# float32r would stream the PE in one pass (vs fp32's two half-speed
# passes) but walrus requires the full producer chain to round to f32r
# and rejects this program; kept off.
USE_F32R = False

_CACHE = {}


def _build_nc():
    import concourse.bacc as bacc
    import concourse.mybir as mybir
    import concourse.tile as tile

    f32 = mybir.dt.float32
    f32mm = mybir.dt.float32r if USE_F32R else f32
    nc = bacc.Bacc("TRN2", target_bir_lowering=False, debug=False)

    art = nc.dram_tensor("article", [BC, AW, DIM], f32, kind="ExternalInput")
    opt = nc.dram_tensor("options", [BC, OW, DIM], f32, kind="ExternalInput")
    sel_a = nc.dram_tensor("sel_a", [P, 2 * BC - 1], f32mm, kind="ExternalInput")
    sel_o = nc.dram_tensor("sel_o", [P, BC], f32mm, kind="ExternalInput")
    out = nc.dram_tensor("out", [BC, 2 * DIM], f32, kind="ExternalOutput")

    # [BC, 128, 16, 300]: partition p <- words p*16 .. p*16+15 (contiguous)
    art_r = art.ap().rearrange("b (p w) f -> b p w f", p=P)
    # per-partition word views: [128, 16, 300] per batch
    art_pw = [art.ap()[b].rearrange("(p w) f -> p w f", p=P) for b in range(BC)]
    # [128, 16, 300]: partition p <- 16 consecutive words of batch p//4
    opt_r = opt.ap().rearrange("b (s q) f -> (b s) q f", s=P // BC)

    with tile.TileContext(nc) as tc:
        with (
            tc.tile_pool(name="const", bufs=1) as cpool,
            tc.tile_pool(name="data", bufs=DATA_BUFS) as dpool,
            tc.tile_pool(name="fold", bufs=FOLD_BUFS) as fpool,
            tc.tile_pool(name="outp", bufs=1) as opool,
            tc.tile_pool(name="psum", bufs=1, space="PSUM") as ppool,
        ):
            # data triggers first so the HBM stream starts at the earliest
            # possible DIRECT2D slot; sel loads are tiny and not needed
            # until the first matmul (~13us in)
            t_opt = dpool.tile([P, AWP, DIM], f32, tag="data")
            nc.sync.dma_start(t_opt[:], opt_r)
            t_b0 = dpool.tile([P, AWP, DIM], f32, tag="data")
            nc.sync.dma_start(t_b0[:], art_r[0])
            # NOTE: every dma_start in this kernel goes through nc.sync.
            # Triggering any DMA from another engine queue (Activation,
            # Vector, ...) puts it on a second hardware ring and measurably
            # degrades Q_I descriptor processing on ALL 16 queues for the
            # whole run (+20% on some queues; ~47us end-to-end).
            sel_a_t = cpool.tile([P, 2 * BC - 1], f32mm, tag="sel_a")
            nc.sync.dma_start(sel_a_t[:], sel_a.ap()[:])
            sel_o_t = cpool.tile([P, BC], f32mm, tag="sel_o")
            nc.sync.dma_start(sel_o_t[:], sel_o.ap()[:])

            psum_a = ppool.tile([BC, DIM], f32, tag="psum_a")
            psum_b = ppool.tile([BC, DIM], f32, tag="psum_b")
            psum_w = ppool.tile([BC, 2 * BC - 1], f32, tag="psum_w")

            # PE warmup: flip the HAM clock gate to 2.4 GHz before the
            # first data tile lands. Results are never read.
            for _ in range(WARMUP_MMS):
                nc.tensor.matmul(
                    psum_w[:], sel_o_t[:], sel_a_t[:], start=True, stop=True
                )

            out_t = opool.tile([BC, 2 * DIM], f32, tag="out")

            def reduce_block(src_ap, nch, sel_ap, psum, first, last,
                             fold_k=FOLD_K, t=None):
                if t is None:
                    t = dpool.tile([P, nch, DIM], f32, tag="data")
                    nc.sync.dma_start(t[:], src_ap)
                cur, n = t, nch
                for lvl in range(fold_k):
                    if n == 1:
                        break
                    n //= 2
                    nxt = fpool.tile([P, n, DIM], f32, tag=f"fold{lvl}_{nch}")
                    nc.vector.tensor_add(nxt[:], cur[:, 0:n, :], cur[:, n : 2 * n, :])
                    cur = nxt
                for j in range(n):
                    nc.tensor.matmul(
                        psum[:],
                        sel_ap,
                        cur[:, j, :],
                        start=(first and j == 0),
                        stop=(last and j == n - 1),
                    )

            # options first; drain its psum into the output tile early
            # (selector columns carry 1/n, so PSUM already holds the mean)
            reduce_block(None, AWP, sel_o_t[:], psum_b, True, True, t=t_opt)
            nc.vector.tensor_copy(out_t[:, DIM : 2 * DIM], psum_b[:])

            for b in range(BC - 3):
                reduce_block(
                    None if b == 0 else art_r[b],
                    AWP,
                    sel_a_t[:, BC - 1 - b : 2 * BC - 1 - b],
                    psum_a,
                    b == 0,
                    False,
                    fold_k=TAPER_FOLD_K if b >= TAPER_START else FOLD_K,
                    t=t_b0 if b == 0 else None,
                )
            # final batches in shrinking tiles -> the very last DMAs are
            # tiny and their fold+matmul tails are short
            n_pieces = len(TAIL_PIECES)
            for i, (b, w0, nch, fk) in enumerate(TAIL_PIECES):
                reduce_block(
                    art_pw[b][:, w0 : w0 + nch, :],
                    nch,
                    sel_a_t[:, BC - 1 - b : 2 * BC - 1 - b],
                    psum_a,
                    False,
                    i == n_pieces - 1,
                    fold_k=fk,
                )

            nc.vector.tensor_copy(out_t[:, 0:DIM], psum_a[:])
            nc.sync.dma_start(out.ap()[:], out_t[:])

    nc.compile()
    return nc


def get_nc():
    if "nc" not in _CACHE:
        _CACHE["nc"] = _build_nc()
    return _CACHE["nc"]


def _sel_arrays():
    # selector columns carry the mean scale (exact powers of two)
    sel_a = np.zeros((P, 2 * BC - 1), np.float32)
    sel_a[:, BC - 1] = 1.0 / AW
    sel_o = np.zeros((P, BC), np.float32)
    sel_o[np.arange(P), np.arange(P) // (P // BC)] = 1.0 / OW
    return sel_a, sel_o


def make_in_maps(article, options):
    article = np.ascontiguousarray(np.asarray(article, dtype=np.float32))
    options = np.ascontiguousarray(np.asarray(options, dtype=np.float32))
    assert article.shape == (B, AW, DIM), article.shape
    assert options.shape == (B, OW, DIM), options.shape
    sel_a, sel_o = _sel_arrays()
    return [
        {
            "article": article[i * BC : (i + 1) * BC],
            "options": options[i * BC : (i + 1) * BC],
            "sel_a": sel_a,
            "sel_o": sel_o,
        }
        for i in range(N_CORES)
    ]


def run_sharded(article, options, **spmd_kwargs):
    from concourse.bass_utils import run_bass_kernel_spmd

    nc = get_nc()
    in_maps = make_in_maps(article, options)
    res = run_bass_kernel_spmd(nc, in_maps, list(range(N_CORES)), **spmd_kwargs)
    full = np.concatenate(
        [res.results[i]["out"] for i in range(N_CORES)], axis=0
    ).astype(np.float32)
    return full, res


def kernel(article_concat, options_concat):
    full, _ = run_sharded(article_concat, options_concat)
    return full

